# revision 41
# baseline (speedup 1.0000x reference)
"""Trainium2 Bass kernel for GPT2Attention with soft-threshold pruning.

Shapes: hidden_states [1, 2048, 1024], H=16 heads, head_dim=64.
Sharding: 2 heads per core across 8 cores (head parallel); c_attn columns and
c_proj rows split by head group; partial c_proj outputs summed on host.

Math per reference (no 1/sqrt(d) scaling):
    w   = q @ k^T                       (causal-masked to C=-1e4)
    w'  = C + (w - C) * sigmoid(10 w)
    a   = softmax(w', axis=-1)
    out = (a @ v) merged -> @ c_proj + b

Device-side we use the shifted score  w'' = w' - C = (w + 1e4)*sigmoid(10w),
which is 0 for masked entries. Softmax over the full row equals
exp(w''-m) / (sum_valid exp(w''-m) + n_masked*exp(-m)) with m = rowmax(w'').
exp(-m) underflows to 0 in fp32 when m > 88, so the masked-tail correction is
only applied for query block 0 (the only place all-pruned rows occur).

Perf structure (vs the fp32/PE-transpose baseline):
  - hs/weights are cast to bf16 on the host; QKV/AV/c_proj matmuls run in
    bf16, score matmuls in fp32r over 512-wide chunks (all 1 cyc/row).
  - the +1e4 shift is folded into the score matmul via 64 aux contraction
    rows (32x256 + 32x56.5, exact in bf16), so dps = w + 1e4 directly.
  - all transposes (hsT, pexp->pT, stat rows) go through the DMA XBAR
    (dma_start_transpose), batched into one instruction each (~1.3us fixed
    dispatch cost per DMA-transpose regardless of size).
  - scalar engine runs only Sigmoid and Exp, both heads merged per
    instruction, grouped over IGROUP query blocks to amortize ACT table
    loads.
  - rowmax uses a stride-4 subsample for blocks i>=1 (any m' within ~80 of
    the true max is exact after normalization; the subsample misses every
    surviving entry with probability < 1e-10 per row). negate=True gives -m
    directly.
  - AV uses the v-stationary form out.T[d,q] = sum_k v[k,d] * p[q,k] with
    both heads' pT concatenated along free (256 wide): half the LDWEIGHTS,
    and the output lands already transposed for c_proj. Normalization is a
    per-column multiply using a DMA-transposed + partition-broadcast recip
    row.
"""

import os
import sys

for _p in ("/opt/trn_rl_repo", "/root/.axon_site/_ro/trn_rl_repo"):
    if os.path.isdir(_p) and _p not in sys.path:
        sys.path.insert(0, _p)

import numpy as np

import concourse.bass as bass
import concourse.tile as tile
from concourse import bacc, mybir

F32 = mybir.dt.float32
F32R = mybir.dt.float32r
BF16 = mybir.dt.bfloat16
F16 = mybir.dt.float16
AF = mybir.ActivationFunctionType
ALU = mybir.AluOpType

S = 2048          # sequence length
D = 1024          # model dim
H = 16            # heads
HD = 64           # head dim
P = 128           # partitions
NB = S // P       # 16 seq blocks
NCORES = 8
HPC = H // NCORES  # 2 heads per core
SLOPE = 10.0
# +1e4 shift folded into the score matmul via 64 aux contraction rows:
# qt rows 64:128 = 1.0; kt rows 64:96 = 256.0, rows 96:128 = 56.5.
# 32*256 + 32*56.5 = 10000 exactly, and both constants are exact in bf16.
CSH_A = 256.0
CSH_B = 56.5
IGROUP = 4         # query blocks per scalar-table group

_CACHE = {}


def _build_nc():
    nc = bacc.Bacc(None, target_bir_lowering=False)

    hs_d = nc.dram_tensor("hs", [S, D], BF16, kind="ExternalInput")
    wqkv_d = nc.dram_tensor("wqkv", [D, 3 * P], BF16, kind="ExternalInput")
    bq_d = nc.dram_tensor("bq", [P, 1], F32, kind="ExternalInput")
    bk_d = nc.dram_tensor("bk", [P, 1], F32, kind="ExternalInput")
    bv_d = nc.dram_tensor("bv", [1, P], BF16, kind="ExternalInput")
    wp_d = nc.dram_tensor("wp", [P, D], BF16, kind="ExternalInput")
    out_d = nc.dram_tensor("out", [S, D], F16, kind="ExternalOutput")

    with tile.TileContext(nc) as tc:
        with (
            tc.tile_pool(name="const", bufs=1) as cpool,
            tc.tile_pool(name="qkt", bufs=1) as qkpool,
            tc.tile_pool(name="psmm", bufs=2, space="PSUM") as ps_mm,
            tc.tile_pool(name="psacc", bufs=2, space="PSUM") as ps_acc,
            tc.tile_pool(name="psout", bufs=2, space="PSUM") as ps_out,
        ):
            ones_p = cpool.tile([P, 1], BF16)   # ones along partitions
            nc.vector.memset(ones_p, 1.0)
            ones_f = cpool.tile([1, 512], BF16)  # ones along free
            nc.vector.memset(ones_f, 1.0)
            sgbias = cpool.tile([P, 1], F32)    # +SLOPE * 1e4 for e_t arg
            nc.vector.memset(sgbias, 1e5)
            mbias = cpool.tile([P, 1], F32)     # -1e4: constant m' for i >= 1
            nc.vector.memset(mbias, -1e4)

            bq_sb = cpool.tile([P, 1], F32)
            nc.sync.dma_start(bq_sb, bq_d[:])
            bk_sb = cpool.tile([P, 1], F32)
            nc.sync.dma_start(bk_sb, bk_d[:])
            bv_sb = cpool.tile([1, P], BF16)
            nc.sync.dma_start(bv_sb, bv_d[:])
            w_bf = cpool.tile([P, D // P, 3 * P], BF16)
            nc.sync.dma_start(w_bf, wqkv_d.rearrange("(o p) f -> p o f", p=P))
            wp_bf = cpool.tile([P, D], BF16)
            nc.sync.dma_start(wp_bf, wp_d[:])

            # persistent per-core tensors
            # qt/kt: [d(64) + 64 aux rows, s]; aux rows implement +1e4
            qt = [qkpool.tile([P, S], F32R, name=f"qt{h}") for h in range(HPC)]
            kt = [qkpool.tile([P, S], F32R, name=f"kt{h}") for h in range(HPC)]
            for t in qt:
                nc.vector.memset(t[HD:P, :].bitcast(F32), 1.0)
            for t in kt:
                nc.vector.memset(t[HD : HD + 32, :].bitcast(F32), CSH_A)
                nc.vector.memset(t[HD + 32 : P, :].bitcast(F32), CSH_B)
            v_sb = qkpool.tile([P, NB, P], BF16)       # V: [k-part, blk, 2*HD]
            ssuf0T = qkpool.tile([P, P], F16)          # row 0: sum_{k>=128} V[k]

            # ---- Interleaved: per 4-seq-block chunk, do the hs load, XBAR
            # transpose and QKV projections, then immediately run the
            # attention blocks whose k-range that chunk completes. Block i=0
            # runs last because its masked-tail correction needs ssuf0T.
            with (
                tc.tile_pool(name="hst", bufs=1) as hstpool,
                tc.tile_pool(name="ws", bufs=1) as wspool,
                tc.tile_pool(name="pexp", bufs=1) as ppool,
                tc.tile_pool(name="sig", bufs=3) as sgpool,
                tc.tile_pool(name="ptsb", bufs=2) as ptpool,
                tc.tile_pool(name="stats", bufs=3) as stpool,
                tc.tile_pool(name="outsb", bufs=2) as opool,
            ):
                hsT = hstpool.tile([P, 4, D // P, 512], BF16)

                def phase_b_chunk(ch):
                    # hsT[p, ch, dc, f] = hs[512*ch + f, dc*128 + p]
                    # (XBAR transpose straight from DRAM, no staging load)
                    nc.sync.dma_start_transpose(
                        hsT[:, ch, :, :],
                        hs_d[512 * ch : 512 * (ch + 1), :],
                    )
                    # QT / KT for this chunk (sc == ch)
                    sc = ch
                    for which, dst, b_ap in (("q", qt, bq_sb), ("k", kt, bk_sb)):
                        off = 0 if which == "q" else P
                        qp = ps_mm.tile([P, 512], F32, tag="mm")
                        for dc in range(D // P):
                            nc.tensor.matmul(
                                qp,
                                lhsT=w_bf[:, dc, off : off + P],
                                rhs=hsT[:, sc, dc, :],
                                start=(dc == 0),
                                stop=(dc == D // P - 1),
                            )
                        for h in range(HPC):
                            nc.vector.tensor_scalar_add(
                                dst[h][:HD, 512 * sc : 512 * (sc + 1)],
                                qp[HD * h : HD * (h + 1)],
                                b_ap[HD * h : HD * (h + 1)],
                            )
                    # V for this chunk: compute vT[d, s] (one 512-wide
                    # matmul chain), then XBAR-transpose into v_sb[s, blk, d]
                    vtp = ps_mm.tile([P, 512], F32, tag="mm")
                    for dc in range(D // P):
                        nc.tensor.matmul(
                            vtp,
                            lhsT=w_bf[:, dc, 2 * P : 3 * P],
                            rhs=hsT[:, ch, dc, :],
                            start=(dc == 0),
                            stop=False,
                        )
                    nc.tensor.matmul(
                        vtp, lhsT=bv_sb, rhs=ones_f, start=False, stop=True
                    )
                    vt_sb = hstpool.tile([P, 512], BF16, name="vt_sb", tag="vt")
                    nc.scalar.copy(vt_sb, vtp)
                    nc.sync.dma_start_transpose(
                        v_sb[:, 4 * ch : 4 * (ch + 1), :], vt_sb
                    )

                def suffix_sum_v():
                    # ssuf0T row 0 = sum over blocks 1..15 of V (as [1,128] row)
                    vs_ps = ps_out.tile([P, 1], F32, tag="po")
                    for sb in range(1, NB):
                        nc.tensor.matmul(
                            vs_ps,
                            lhsT=v_sb[:, sb, :],
                            rhs=ones_p,
                            start=(sb == 1),
                            stop=(sb == NB - 1),
                        )
                    vpad = cpool.tile([P, P], F16)
                    nc.vector.tensor_copy(vpad[:, 0:1], vs_ps)
                    nc.sync.dma_start_transpose(ssuf0T, vpad)

                # All scalar activations use the 'exp_and_others' ACT table
                # (exp + identity + copy): one table load for the whole kernel.
                # sigma(10w) comes from e_t = exp(-10w): nws = (e_t-1)*dps.
                ws_t = {}

                def attn_stage_a(i):
                    W = P * (i + 1)
                    NC = (W + 511) // 512
                    qsl = slice(P * i, P * (i + 1))
                    wsp = wspool.tile(
                        [P, HPC, S], F32, tag=f"ws{2 if i == 0 else i % 2}"
                    )
                    ws_t[i] = wsp
                    for c in range(NC):
                        off = 512 * c
                        cw = min(512, W - off)  # valid width
                        dps = ps_mm.tile([P, HPC * 512], F32, tag="mm")
                        for h in range(HPC):
                            nc.tensor.matmul(
                                dps[:, 512 * h : 512 * (h + 1)],
                                lhsT=qt[h][:, qsl],
                                rhs=kt[h][:, off : off + 512],
                                start=True,
                                stop=True,
                            )
                        dpv = dps.rearrange("p (h c) -> p h c", h=HPC)
                        et = sgpool.tile([P, HPC, 512], F32, tag="sig")
                        # e_t = exp(-10*(dps-1e4)); sigma = 1/(1+e_t)
                        nc.scalar.activation(
                            et[:, :, :cw],
                            dpv[:, :, :cw],
                            AF.Exp,
                            scale=-SLOPE,
                            bias=sgbias,
                        )
                        if c == NC - 1:
                            # mask above the diagonal so nws = 0 there
                            nc.gpsimd.affine_select(
                                out=et[:, :, :cw],
                                in_=et[:, :, :cw],
                                pattern=[[0, HPC], [-1, cw]],
                                channel_multiplier=1,
                                base=P * i - off,
                                compare_op=ALU.is_ge,
                                fill=(1e30 if i == 0 else 1.0),
                            )
                        if i == 0:
                            # exact sigma = 1/(1+e_t): block 0 has all-pruned
                            # rows whose small positive w'' still matters
                            nc.vector.tensor_scalar_add(
                                et[:, :, :cw], et[:, :, :cw], 1.0
                            )
                            nc.vector.reciprocal(et[:, :, :cw], et[:, :, :cw])
                            nc.vector.scalar_tensor_tensor(
                                out=wsp[:, :, off : off + cw],
                                in0=et[:, :, :cw],
                                scalar=-1.0,
                                in1=dpv[:, :, :cw],
                                op0=ALU.mult,
                                op1=ALU.mult,
                            )
                        else:
                            # nws = (e_t - 1) * dps = -(w+1e4)*sigma(10w)
                            # (exact wherever exp(w''-m) > 0)
                            nc.vector.scalar_tensor_tensor(
                                out=wsp[:, :, off : off + cw],
                                in0=et[:, :, :cw],
                                scalar=1.0,
                                in1=dpv[:, :, :cw],
                                op0=ALU.subtract,
                                op1=ALU.mult,
                            )

                def attn_stage_b(i):
                    W = P * (i + 1)
                    wsp = ws_t.pop(i)
                    mn_t = {}
                    sm_t = {}
                    for h in range(HPC):
                        if i == 0:
                            # exact -rowmax(w''): block 0 has all-pruned rows
                            mn = stpool.tile([P, 1], F32, tag=f"mn{h}")
                            nc.vector.tensor_reduce(
                                mn, wsp[:, h, :W], mybir.AxisListType.X, ALU.min
                            )
                            mn_t[h] = mn
                        else:
                            # constant m' = 1e4: within ~80 of the true max
                            # whenever rowmax(w) >= -80, which holds w.p. ~1;
                            # any such m' is exact after normalization
                            mn_t[h] = mbias
                    pexp = ppool.tile(
                        [P, HPC * S], BF16, tag=f"pe{2 if i == 0 else i % 2}"
                    )
                    for h in range(HPC):
                        sm = stpool.tile([P, 1], F32, tag=f"sm{i % 2}{h}")
                        # pexp = exp(w'' - m) = exp(-nws + mn)
                        nc.scalar.activation(
                            pexp[:, h * W : (h + 1) * W],
                            wsp[:, h, :W],
                            AF.Exp,
                            scale=-1.0,
                            bias=mn_t[h],
                            accum_out=sm,
                        )
                        sm_t[h] = sm
                        if i == 0:
                            e_sb = stpool.tile([P, 1], F32, tag=f"e{h}")
                            nc.scalar.activation(e_sb, mn_t[h], AF.Exp)
                            sm_t["e", h] = e_sb
                    # Normalization trick: rescale h1's pexp by d0/d1 so both
                    # heads share denominator d0, which is then divided out for
                    # free in the y copy (Identity activation with per-q scale).
                    o_ps = ps_acc.tile([P, 2 * P], F32, tag="acc")
                    den = {}
                    for h in range(HPC):
                        if i == 0:
                            d0t = stpool.tile([P, 1], F32, tag=f"d0{h}")
                            nc.vector.tensor_scalar_mul(
                                d0t, sm_t["e", h], float(S - P)
                            )
                            nc.vector.tensor_add(d0t, d0t, sm_t[h])
                            den[h] = d0t
                        else:
                            den[h] = sm_t[h]
                    r0 = stpool.tile([P, 1], F32, tag="r0")
                    nc.vector.reciprocal(r0, den[0])
                    r1 = stpool.tile([P, 1], F32, tag="r1")
                    nc.vector.reciprocal(r1, den[1])
                    alpha = stpool.tile([P, 1], F32, tag="al")
                    nc.vector.tensor_tensor(
                        out=alpha, in0=den[0], in1=r1, op=ALU.mult
                    )
                    nc.vector.tensor_scalar_mul(
                        pexp[:, W : 2 * W], pexp[:, W : 2 * W], alpha
                    )
                    ptsb = ptpool.tile([P, 2 * NB, P], BF16, tag="pt")
                    nc.sync.dma_start_transpose(
                        ptsb[:, : 2 * (i + 1), :], pexp[:, : 2 * W]
                    )
                    pt4 = ptsb[:, : 2 * (i + 1), :].rearrange(
                        "p (h j) f -> p h j f", h=HPC
                    )
                    for j in range(i + 1):
                        nc.tensor.matmul(
                            o_ps,
                            lhsT=v_sb[:, j, :],
                            rhs=pt4[:, :, j, :],
                            start=(j == 0),
                            stop=(j == i and i > 0),
                        )
                    if i == 0:
                        # masked-tail: o.T[d, q] += e'_h[q] * ssuf0[d], where
                        # e'_h0 = e_h0 and e'_h1 = e_h1 * alpha (common den d0)
                        for h in range(HPC):
                            ep = stpool.tile([P, P], F16, tag=f"ep{h}")
                            if h == 0:
                                nc.vector.tensor_copy(ep[:, 0:1], sm_t["e", h])
                            else:
                                ea = stpool.tile([P, 1], F32, tag="ea")
                                nc.vector.tensor_tensor(
                                    out=ea, in0=sm_t["e", h], in1=alpha,
                                    op=ALU.mult,
                                )
                                nc.vector.tensor_copy(ep[:, 0:1], ea)
                            eT = stpool.tile([P, P], F16, tag=f"eT{h}")
                            nc.sync.dma_start_transpose(eT, ep)
                            nc.tensor.matmul(
                                o_ps[:, P * h : P * (h + 1)],
                                lhsT=ssuf0T[0:1, :],
                                rhs=eT[0:1, :],
                                start=False,
                                stop=(h == HPC - 1),
                                skip_group_check=True,
                            )
                    # merge heads: ot[d, q] (already normalized)
                    ot_sb = opool.tile([P, P], BF16, tag="ot")
                    for h in range(HPC):
                        nc.vector.tensor_copy(
                            ot_sb[HD * h : HD * (h + 1), :],
                            o_ps[HD * h : HD * (h + 1), P * h : P * (h + 1)],
                        )
                    # c_proj partial for this query block (f16 halves the
                    # output store bytes; partials are summed on host)
                    y_sb = opool.tile([P, D], F16, tag="y")
                    for nch in range(D // 512):
                        yp = ps_out.tile([P, 512], F32, tag="po")
                        nc.tensor.matmul(
                            yp,
                            lhsT=ot_sb,
                            rhs=wp_bf[:, 512 * nch : 512 * (nch + 1)],
                            start=True,
                            stop=True,
                        )
                        # y = yp / d0 (per-partition scale); split between
                        # scalar (Identity is in the exp ACT table) and vector
                        if nch == 0:
                            nc.scalar.activation(
                                y_sb[:, 512 * nch : 512 * (nch + 1)],
                                yp,
                                AF.Identity,
                                scale=r0,
                            )
                        else:
                            nc.vector.tensor_scalar_mul(
                                y_sb[:, 512 * nch : 512 * (nch + 1)], yp, r0
                            )
                    nc.sync.dma_start(out_d[P * i : P * (i + 1), :], y_sb)

                queue = list(range(1, NB)) + [0]
                prev = None
                for k, i in enumerate(queue):
                    ch = i // 4
                    if ch * 4 + (1 if ch == 0 else 0) == i:
                        phase_b_chunk(ch)
                    if i == 0:
                        suffix_sum_v()
                    attn_stage_a(i)
                    if prev is not None:
                        attn_stage_b(prev)
                    prev = i
                attn_stage_b(prev)

    nc.compile()
    return nc


def _get_nc():
    if "nc" not in _CACHE:
        _CACHE["nc"] = _build_nc()
    return _CACHE["nc"]


def kernel(hidden_states, c_attn_w, c_attn_b, c_proj_w, c_proj_b):
    import ml_dtypes
    from concourse.bass_utils import run_bass_kernel_spmd

    BF = ml_dtypes.bfloat16
    hs = np.ascontiguousarray(
        np.asarray(hidden_states, np.float32).reshape(S, D).astype(BF)
    )
    caw = np.asarray(c_attn_w, np.float32)
    cab = np.asarray(c_attn_b, np.float32)
    cpw = np.asarray(c_proj_w, np.float32)
    cpb = np.asarray(c_proj_b, np.float32)

    in_maps = []
    for c in range(NCORES):
        heads = [HPC * c + h for h in range(HPC)]
        qcols = [caw[:, HD * h : HD * (h + 1)] for h in heads]
        kcols = [caw[:, D + HD * h : D + HD * (h + 1)] for h in heads]
        vcols = [caw[:, 2 * D + HD * h : 2 * D + HD * (h + 1)] for h in heads]
        wqkv = np.ascontiguousarray(
            np.concatenate(qcols + kcols + vcols, axis=1).astype(BF)
        )
        bq = np.concatenate([cab[HD * h : HD * (h + 1)] for h in heads])
        bk = np.concatenate([cab[D + HD * h : D + HD * (h + 1)] for h in heads])
        bv = np.concatenate([cab[2 * D + HD * h : 2 * D + HD * (h + 1)] for h in heads])
        wp = np.ascontiguousarray(cpw[P * c : P * (c + 1), :].astype(BF))
        in_maps.append(
            {
                "hs": hs,
                "wqkv": wqkv,
                "bq": np.ascontiguousarray(bq.reshape(P, 1)).astype(np.float32),
                "bk": np.ascontiguousarray(bk.reshape(P, 1)).astype(np.float32),
                "bv": np.ascontiguousarray(bv.reshape(1, P)).astype(BF),
                "wp": wp,
            }
        )

    nc = _get_nc()
    res = run_bass_kernel_spmd(nc, in_maps, core_ids=list(range(NCORES)))
    out = np.zeros((S, D), np.float64)
    for c in range(NCORES):
        out += res.results[c]["out"].astype(np.float64)
    out = out.astype(np.float32) + cpb[None, :].astype(np.float32)
    return out.reshape(1, S, D)


# revision 42
# speedup vs baseline: 1.0772x; 1.0772x over previous
"""Trainium2 Bass kernel for GPT2Attention with soft-threshold pruning.

Shapes: hidden_states [1, 2048, 1024], H=16 heads, head_dim=64.
Sharding: 2 heads per core across 8 cores (head parallel); c_attn columns and
c_proj rows split by head group; partial c_proj outputs summed on host.

Math per reference (no 1/sqrt(d) scaling):
    w   = q @ k^T                       (causal-masked to C=-1e4)
    w'  = C + (w - C) * sigmoid(10 w)
    a   = softmax(w', axis=-1)
    out = (a @ v) merged -> @ c_proj + b

Device-side we use the shifted score  w'' = w' - C = (w + 1e4)*sigmoid(10w),
which is 0 for masked entries. Softmax over the full row equals
exp(w''-m) / (sum_valid exp(w''-m) + n_masked*exp(-m)) with m = rowmax(w'').
exp(-m) underflows to 0 in fp32 when m > 88, so the masked-tail correction is
only applied for query block 0 (the only place all-pruned rows occur).

Perf structure (vs the fp32/PE-transpose baseline):
  - hs/weights are cast to bf16 on the host; QKV/AV/c_proj matmuls run in
    bf16, score matmuls in fp32r over 512-wide chunks (all 1 cyc/row).
  - the +1e4 shift is folded into the score matmul via 64 aux contraction
    rows (32x256 + 32x56.5, exact in bf16), so dps = w + 1e4 directly.
  - all transposes (hsT, pexp->pT, stat rows) go through the DMA XBAR
    (dma_start_transpose), batched into one instruction each (~1.3us fixed
    dispatch cost per DMA-transpose regardless of size).
  - scalar engine runs only Sigmoid and Exp, both heads merged per
    instruction, grouped over IGROUP query blocks to amortize ACT table
    loads.
  - rowmax uses a stride-4 subsample for blocks i>=1 (any m' within ~80 of
    the true max is exact after normalization; the subsample misses every
    surviving entry with probability < 1e-10 per row). negate=True gives -m
    directly.
  - AV uses the v-stationary form out.T[d,q] = sum_k v[k,d] * p[q,k] with
    both heads' pT concatenated along free (256 wide): half the LDWEIGHTS,
    and the output lands already transposed for c_proj. Normalization is a
    per-column multiply using a DMA-transposed + partition-broadcast recip
    row.
"""

import os
import sys

for _p in ("/opt/trn_rl_repo", "/root/.axon_site/_ro/trn_rl_repo"):
    if os.path.isdir(_p) and _p not in sys.path:
        sys.path.insert(0, _p)

import numpy as np

import concourse.bass as bass
import concourse.tile as tile
from concourse import bacc, mybir

F32 = mybir.dt.float32
F32R = mybir.dt.float32r
BF16 = mybir.dt.bfloat16
F16 = mybir.dt.float16
AF = mybir.ActivationFunctionType
ALU = mybir.AluOpType

S = 2048          # sequence length
D = 1024          # model dim
H = 16            # heads
HD = 64           # head dim
P = 128           # partitions
NB = S // P       # 16 seq blocks
NCORES = 8
HPC = H // NCORES  # 2 heads per core
SLOPE = 10.0
# +1e4 shift folded into the score matmul via 64 aux contraction rows:
# qt rows 64:128 = 1.0; kt rows 64:96 = 256.0, rows 96:128 = 56.5.
# 32*256 + 32*56.5 = 10000 exactly, and both constants are exact in bf16.
CSH_A = 256.0
CSH_B = 56.5
IGROUP = 4         # query blocks per scalar-table group

_CACHE = {}


def _build_nc():
    nc = bacc.Bacc(None, target_bir_lowering=False)

    hs_d = nc.dram_tensor("hs", [S, D], BF16, kind="ExternalInput")
    wqkv_d = nc.dram_tensor("wqkv", [D, 3 * P], BF16, kind="ExternalInput")
    bq_d = nc.dram_tensor("bq", [P, 1], F32, kind="ExternalInput")
    bk_d = nc.dram_tensor("bk", [P, 1], F32, kind="ExternalInput")
    bv_d = nc.dram_tensor("bv", [1, P], BF16, kind="ExternalInput")
    wp_d = nc.dram_tensor("wp", [P, D], BF16, kind="ExternalInput")
    out_d = nc.dram_tensor("out", [S, D], F16, kind="ExternalOutput")

    with tile.TileContext(nc) as tc:
        with (
            tc.tile_pool(name="const", bufs=1) as cpool,
            tc.tile_pool(name="qkt", bufs=1) as qkpool,
            tc.tile_pool(name="psmm", bufs=2, space="PSUM") as ps_mm,
            tc.tile_pool(name="psacc", bufs=2, space="PSUM") as ps_acc,
            tc.tile_pool(name="psout", bufs=2, space="PSUM") as ps_out,
        ):
            ones_p = cpool.tile([P, 1], BF16)   # ones along partitions
            nc.vector.memset(ones_p, 1.0)
            ones_f = cpool.tile([1, P], BF16)   # ones along free
            nc.vector.memset(ones_f, 1.0)
            sgbias = cpool.tile([P, 1], F32)    # +SLOPE * 1e4 for e_t arg
            nc.vector.memset(sgbias, 1e5)
            mbias = cpool.tile([P, 1], F32)     # -1e4: constant m' for i >= 1
            nc.vector.memset(mbias, -1e4)

            bq_sb = cpool.tile([P, 1], F32)
            nc.sync.dma_start(bq_sb, bq_d[:])
            bk_sb = cpool.tile([P, 1], F32)
            nc.sync.dma_start(bk_sb, bk_d[:])
            bv_sb = cpool.tile([1, P], BF16)
            nc.sync.dma_start(bv_sb, bv_d[:])
            w_bf = cpool.tile([P, D // P, 3 * P], BF16)
            nc.sync.dma_start(w_bf, wqkv_d.rearrange("(o p) f -> p o f", p=P))
            wp_bf = cpool.tile([P, D], BF16)
            nc.sync.dma_start(wp_bf, wp_d[:])

            # persistent per-core tensors
            # qt/kt: [d(64) + 64 aux rows, s]; aux rows implement +1e4
            qt = [qkpool.tile([P, S], F32R, name=f"qt{h}") for h in range(HPC)]
            kt = [qkpool.tile([P, S], F32R, name=f"kt{h}") for h in range(HPC)]
            for t in qt:
                nc.gpsimd.memset(t[HD:P, :].bitcast(F32), 1.0)
            for t in kt:
                nc.gpsimd.memset(t[HD : HD + 32, :].bitcast(F32), CSH_A)
                nc.gpsimd.memset(t[HD + 32 : P, :].bitcast(F32), CSH_B)
            v_sb = qkpool.tile([P, NB, P], BF16)       # V: [k-part, blk, 2*HD]
            ssuf0T = qkpool.tile([P, P], F16)          # row 0: sum_{k>=128} V[k]

            # ---- Interleaved: per 4-seq-block chunk, do the hs load, XBAR
            # transpose and QKV projections, then immediately run the
            # attention blocks whose k-range that chunk completes. Block i=0
            # runs last because its masked-tail correction needs ssuf0T.
            with (
                tc.tile_pool(name="hst", bufs=1) as hstpool,
                tc.tile_pool(name="ws", bufs=1) as wspool,
                tc.tile_pool(name="pexp", bufs=1) as ppool,
                tc.tile_pool(name="sig", bufs=3) as sgpool,
                tc.tile_pool(name="ptsb", bufs=2) as ptpool,
                tc.tile_pool(name="stats", bufs=3) as stpool,
                tc.tile_pool(name="outsb", bufs=2) as opool,
            ):
                hsT = hstpool.tile([P, 4, D // P, 512], BF16)

                def phase_b_chunk(ch):
                    # hsT[p, ch, dc, f] = hs[512*ch + f, dc*128 + p]
                    # (XBAR transpose straight from DRAM, no staging load)
                    nc.sync.dma_start_transpose(
                        hsT[:, ch, :, :],
                        hs_d[512 * ch : 512 * (ch + 1), :],
                    )
                    # QT / KT for this chunk (sc == ch)
                    sc = ch
                    for which, dst, b_ap in (("q", qt, bq_sb), ("k", kt, bk_sb)):
                        off = 0 if which == "q" else P
                        qp = ps_mm.tile([P, 512], F32, tag="mm")
                        for dc in range(D // P):
                            nc.tensor.matmul(
                                qp,
                                lhsT=w_bf[:, dc, off : off + P],
                                rhs=hsT[:, sc, dc, :],
                                start=(dc == 0),
                                stop=(dc == D // P - 1),
                            )
                        for h in range(HPC):
                            nc.vector.tensor_scalar_add(
                                dst[h][:HD, 512 * sc : 512 * (sc + 1)],
                                qp[HD * h : HD * (h + 1)],
                                b_ap[HD * h : HD * (h + 1)],
                            )
                    # V for this chunk's 4 seq blocks
                    for sb in range(4 * ch, 4 * (ch + 1)):
                        vp = ps_acc.tile([P, P], F32, tag="acc")
                        for dc in range(D // P):
                            nc.tensor.matmul(
                                vp,
                                lhsT=hsT[:, sb // 4, dc,
                                         P * (sb % 4) : P * (sb % 4 + 1)],
                                rhs=w_bf[:, dc, 2 * P : 3 * P],
                                start=(dc == 0),
                                stop=False,
                            )
                        nc.tensor.matmul(
                            vp, lhsT=ones_f, rhs=bv_sb, start=False, stop=True
                        )
                        nc.vector.tensor_copy(v_sb[:, sb, :], vp)

                def suffix_sum_v():
                    # ssuf0T row 0 = sum over blocks 1..15 of V (as [1,128] row)
                    vs_ps = ps_out.tile([P, 1], F32, tag="po")
                    for sb in range(1, NB):
                        nc.tensor.matmul(
                            vs_ps,
                            lhsT=v_sb[:, sb, :],
                            rhs=ones_p,
                            start=(sb == 1),
                            stop=(sb == NB - 1),
                        )
                    vpad = cpool.tile([P, P], F16)
                    nc.vector.tensor_copy(vpad[:, 0:1], vs_ps)
                    nc.sync.dma_start_transpose(ssuf0T, vpad)

                # All scalar activations use the 'exp_and_others' ACT table
                # (exp + identity + copy): one table load for the whole kernel.
                # sigma(10w) comes from e_t = exp(-10w): nws = (e_t-1)*dps.
                ws_t = {}

                def attn_stage_a(i):
                    W = P * (i + 1)
                    NC = (W + 511) // 512
                    qsl = slice(P * i, P * (i + 1))
                    wsp = wspool.tile([P, HPC, S], F32, tag=f"ws{i % 2}")
                    ws_t[i] = wsp
                    for c in range(NC):
                        off = 512 * c
                        cw = min(512, W - off)  # valid width
                        dps = ps_mm.tile([P, HPC * 512], F32, tag="mm")
                        for h in range(HPC):
                            nc.tensor.matmul(
                                dps[:, 512 * h : 512 * (h + 1)],
                                lhsT=qt[h][:, qsl],
                                rhs=kt[h][:, off : off + 512],
                                start=True,
                                stop=True,
                            )
                        dpv = dps.rearrange("p (h c) -> p h c", h=HPC)
                        et = sgpool.tile([P, HPC, 512], F32, tag="sig")
                        # e_t = exp(-10*(dps-1e4)); sigma = 1/(1+e_t)
                        nc.scalar.activation(
                            et[:, :, :cw],
                            dpv[:, :, :cw],
                            AF.Exp,
                            scale=-SLOPE,
                            bias=sgbias,
                        )
                        if c == NC - 1:
                            # mask above the diagonal so nws = 0 there
                            nc.gpsimd.affine_select(
                                out=et[:, :, :cw],
                                in_=et[:, :, :cw],
                                pattern=[[0, HPC], [-1, cw]],
                                channel_multiplier=1,
                                base=P * i - off,
                                compare_op=ALU.is_ge,
                                fill=(1e30 if i == 0 else 1.0),
                            )
                        if i == 0:
                            # exact sigma = 1/(1+e_t): block 0 has all-pruned
                            # rows whose small positive w'' still matters
                            nc.vector.tensor_scalar_add(
                                et[:, :, :cw], et[:, :, :cw], 1.0
                            )
                            nc.vector.reciprocal(et[:, :, :cw], et[:, :, :cw])
                            nc.vector.scalar_tensor_tensor(
                                out=wsp[:, :, off : off + cw],
                                in0=et[:, :, :cw],
                                scalar=-1.0,
                                in1=dpv[:, :, :cw],
                                op0=ALU.mult,
                                op1=ALU.mult,
                            )
                        else:
                            # nws = (e_t - 1) * dps = -(w+1e4)*sigma(10w)
                            # (exact wherever exp(w''-m) > 0)
                            nc.vector.scalar_tensor_tensor(
                                out=wsp[:, :, off : off + cw],
                                in0=et[:, :, :cw],
                                scalar=1.0,
                                in1=dpv[:, :, :cw],
                                op0=ALU.subtract,
                                op1=ALU.mult,
                            )

                def attn_stage_b(i):
                    W = P * (i + 1)
                    wsp = ws_t.pop(i)
                    mn_t = {}
                    sm_t = {}
                    for h in range(HPC):
                        if i == 0:
                            # exact -rowmax(w''): block 0 has all-pruned rows
                            mn = stpool.tile([P, 1], F32, tag=f"mn{h}")
                            nc.vector.tensor_reduce(
                                mn, wsp[:, h, :W], mybir.AxisListType.X, ALU.min
                            )
                            mn_t[h] = mn
                        else:
                            # constant m' = 1e4: within ~80 of the true max
                            # whenever rowmax(w) >= -80, which holds w.p. ~1;
                            # any such m' is exact after normalization
                            mn_t[h] = mbias
                    pexp = ppool.tile([P, HPC * S], BF16, tag=f"pe{i % 2}")
                    for h in range(HPC):
                        sm = stpool.tile([P, 1], F32, tag=f"sm{i % 2}{h}")
                        # pexp = exp(w'' - m) = exp(-nws + mn)
                        nc.scalar.activation(
                            pexp[:, h * W : (h + 1) * W],
                            wsp[:, h, :W],
                            AF.Exp,
                            scale=-1.0,
                            bias=mn_t[h],
                            accum_out=sm,
                        )
                        sm_t[h] = sm
                        if i == 0:
                            e_sb = stpool.tile([P, 1], F32, tag=f"e{h}")
                            nc.scalar.activation(e_sb, mn_t[h], AF.Exp)
                            sm_t["e", h] = e_sb
                    # Normalization trick: rescale h1's pexp by d0/d1 so both
                    # heads share denominator d0, which is then divided out for
                    # free in the y copy (Identity activation with per-q scale).
                    o_ps = ps_acc.tile([P, 2 * P], F32, tag="acc")
                    den = {}
                    for h in range(HPC):
                        if i == 0:
                            d0t = stpool.tile([P, 1], F32, tag=f"d0{h}")
                            nc.vector.tensor_scalar_mul(
                                d0t, sm_t["e", h], float(S - P)
                            )
                            nc.vector.tensor_add(d0t, d0t, sm_t[h])
                            den[h] = d0t
                        else:
                            den[h] = sm_t[h]
                    r0 = stpool.tile([P, 1], F32, tag="r0")
                    nc.vector.reciprocal(r0, den[0])
                    r1 = stpool.tile([P, 1], F32, tag="r1")
                    nc.vector.reciprocal(r1, den[1])
                    alpha = stpool.tile([P, 1], F32, tag="al")
                    nc.vector.tensor_tensor(
                        out=alpha, in0=den[0], in1=r1, op=ALU.mult
                    )
                    nc.vector.tensor_scalar_mul(
                        pexp[:, W : 2 * W], pexp[:, W : 2 * W], alpha
                    )
                    ptsb = ptpool.tile([P, 2 * NB, P], BF16, tag="pt")
                    nc.sync.dma_start_transpose(
                        ptsb[:, : 2 * (i + 1), :], pexp[:, : 2 * W]
                    )
                    pt4 = ptsb[:, : 2 * (i + 1), :].rearrange(
                        "p (h j) f -> p h j f", h=HPC
                    )
                    for j in range(i + 1):
                        nc.tensor.matmul(
                            o_ps,
                            lhsT=v_sb[:, j, :],
                            rhs=pt4[:, :, j, :],
                            start=(j == 0),
                            stop=(j == i and i > 0),
                        )
                    if i == 0:
                        # masked-tail: o.T[d, q] += e'_h[q] * ssuf0[d], where
                        # e'_h0 = e_h0 and e'_h1 = e_h1 * alpha (common den d0)
                        for h in range(HPC):
                            ep = stpool.tile([P, P], F16, tag=f"ep{h}")
                            if h == 0:
                                nc.vector.tensor_copy(ep[:, 0:1], sm_t["e", h])
                            else:
                                ea = stpool.tile([P, 1], F32, tag="ea")
                                nc.vector.tensor_tensor(
                                    out=ea, in0=sm_t["e", h], in1=alpha,
                                    op=ALU.mult,
                                )
                                nc.vector.tensor_copy(ep[:, 0:1], ea)
                            eT = stpool.tile([P, P], F16, tag=f"eT{h}")
                            nc.sync.dma_start_transpose(eT, ep)
                            nc.tensor.matmul(
                                o_ps[:, P * h : P * (h + 1)],
                                lhsT=ssuf0T[0:1, :],
                                rhs=eT[0:1, :],
                                start=False,
                                stop=(h == HPC - 1),
                                skip_group_check=True,
                            )
                    # merge heads: ot[d, q] (already normalized)
                    ot_sb = opool.tile([P, P], BF16, tag="ot")
                    for h in range(HPC):
                        nc.vector.tensor_copy(
                            ot_sb[HD * h : HD * (h + 1), :],
                            o_ps[HD * h : HD * (h + 1), P * h : P * (h + 1)],
                        )
                    # c_proj partial for this query block (f16 halves the
                    # output store bytes; partials are summed on host)
                    y_sb = opool.tile([P, D], F16, tag="y")
                    for nch in range(D // 512):
                        yp = ps_out.tile([P, 512], F32, tag="po")
                        nc.tensor.matmul(
                            yp,
                            lhsT=ot_sb,
                            rhs=wp_bf[:, 512 * nch : 512 * (nch + 1)],
                            start=True,
                            stop=True,
                        )
                        # y = yp / d0 (per-partition scale); split between
                        # scalar (Identity is in the exp ACT table) and vector
                        if nch == 0:
                            nc.scalar.activation(
                                y_sb[:, 512 * nch : 512 * (nch + 1)],
                                yp,
                                AF.Identity,
                                scale=r0,
                            )
                        else:
                            nc.vector.tensor_scalar_mul(
                                y_sb[:, 512 * nch : 512 * (nch + 1)], yp, r0
                            )
                    nc.sync.dma_start(out_d[P * i : P * (i + 1), :], y_sb)

                queue = list(range(1, NB)) + [0]
                prev = None
                for k, i in enumerate(queue):
                    ch = i // 4
                    if ch * 4 + (1 if ch == 0 else 0) == i:
                        phase_b_chunk(ch)
                    if i == 0:
                        suffix_sum_v()
                    attn_stage_a(i)
                    if prev is not None:
                        attn_stage_b(prev)
                    prev = i
                attn_stage_b(prev)

    nc.compile()
    return nc


def _get_nc():
    if "nc" not in _CACHE:
        _CACHE["nc"] = _build_nc()
    return _CACHE["nc"]


def kernel(hidden_states, c_attn_w, c_attn_b, c_proj_w, c_proj_b):
    import ml_dtypes
    from concourse.bass_utils import run_bass_kernel_spmd

    BF = ml_dtypes.bfloat16
    hs = np.ascontiguousarray(
        np.asarray(hidden_states, np.float32).reshape(S, D).astype(BF)
    )
    caw = np.asarray(c_attn_w, np.float32)
    cab = np.asarray(c_attn_b, np.float32)
    cpw = np.asarray(c_proj_w, np.float32)
    cpb = np.asarray(c_proj_b, np.float32)

    in_maps = []
    for c in range(NCORES):
        heads = [HPC * c + h for h in range(HPC)]
        qcols = [caw[:, HD * h : HD * (h + 1)] for h in heads]
        kcols = [caw[:, D + HD * h : D + HD * (h + 1)] for h in heads]
        vcols = [caw[:, 2 * D + HD * h : 2 * D + HD * (h + 1)] for h in heads]
        wqkv = np.ascontiguousarray(
            np.concatenate(qcols + kcols + vcols, axis=1).astype(BF)
        )
        bq = np.concatenate([cab[HD * h : HD * (h + 1)] for h in heads])
        bk = np.concatenate([cab[D + HD * h : D + HD * (h + 1)] for h in heads])
        bv = np.concatenate([cab[2 * D + HD * h : 2 * D + HD * (h + 1)] for h in heads])
        wp = np.ascontiguousarray(cpw[P * c : P * (c + 1), :].astype(BF))
        in_maps.append(
            {
                "hs": hs,
                "wqkv": wqkv,
                "bq": np.ascontiguousarray(bq.reshape(P, 1)).astype(np.float32),
                "bk": np.ascontiguousarray(bk.reshape(P, 1)).astype(np.float32),
                "bv": np.ascontiguousarray(bv.reshape(1, P)).astype(BF),
                "wp": wp,
            }
        )

    nc = _get_nc()
    res = run_bass_kernel_spmd(nc, in_maps, core_ids=list(range(NCORES)))
    out = np.zeros((S, D), np.float64)
    for c in range(NCORES):
        out += res.results[c]["out"].astype(np.float64)
    out = out.astype(np.float32) + cpb[None, :].astype(np.float32)
    return out.reshape(1, S, D)
